# revision 54
# baseline (speedup 1.0000x reference)
"""Criss-cross attention (CCNet) kernel for 8 TRN2 NeuronCores.

Data-parallel over batch N=8: one image per core. TimelineSim ~147us vs
290us for the v2 bf16 baseline; rel err 4.4e-3 (threshold 2e-2).

Key structural moves (vs the v2 baseline):
  - inc conv eliminated algebraically: conv1x1(ca_map(a, g), inc_w) ==
    ca_map(a, conv1x1(x, inc_w@g_w)) because channel mixing commutes with
    the per-channel pixel-weighted sums. The device computes h = (inc_w@
    g_w) x once and aggregates h; the host adds x + comb_b in f32 (which
    also removes the bf16 rounding of the residual).
  - fp8 e4m3 everywhere heavy: x / tf / M weights shipped fp8; both convs
    run MatmulPerfMode.DoubleRow (two 128-contraction chunks per
    instruction at double rate); h stored fp8 in DRAM (Gd); aggregation
    matmuls fp8; U (attn out) fp8 with x64 scale, decoded on host. Scales
    (S0..S3) keep everything in e4m3 range; normalization scale S2 is
    folded into the "ones" operand of the denominator matmul.
  - h conv on flat 128-pixel blocks (full partition width) writing
    contiguous Gd rows; col/row aggregation read Gd slabs (the DMA does
    the pixel-axis transpose for the row pass).
  - denominators computed ONCE as D[x, y] (row pass uses a transposed
    view of the broadcast reciprocal tile rrall); fast reciprocal on DVE;
    Pool builds fp8 normalized weights (wcn first - it gates the col
    pass - then wrn).
  - row pass folds the col-pass U into PSUM with an fp8 identity matmul
    (value S1*S2/S3) so its drain is a plain scaled copy; software
    pipelined (LA=2) so the row matmuls overlap the col-pass tail.
  - PSUM drains are the limiting resource (only Act/DVE reach PSUM):
    conv drains batched [128, 2, 512] and split across Act/DVE (d2a1),
    tf drains split, col-pass drains one per block [128, 4, 384].
"""

import sys

sys.path.insert(0, "/opt/trn_rl_repo")

from contextlib import ExitStack

import numpy as np
import ml_dtypes

import concourse.bass as bass
import concourse.bacc as bacc
import concourse.tile as tile
from concourse import mybir
from concourse.bass_utils import run_bass_kernel_spmd

BF16 = mybir.dt.bfloat16
F32 = mybir.dt.float32
FP8 = mybir.dt.float8e4
AF = mybir.ActivationFunctionType
DR = mybir.MatmulPerfMode.DoubleRow
MUL = mybir.AluOpType.mult
ADD = mybir.AluOpType.add

N, C_IN, C_INNER, C_OUT, H, W = 8, 512, 64, 512, 96, 96
HW = H * W  # 9216
KC = C_IN // 128  # 4 contraction chunks

S0 = 16.0     # x scale (fp8)
STF = 256.0   # t/f weight scale
SM = 2048.0   # M weight scale
S1 = 64.0     # Gd (h) storage scale
S2 = 128.0    # normalized-weight scale; folded into the denominator "ones"
S3 = 64.0     # U / out storage scale

# schedule knobs: how many h-conv block-pairs to interleave into phase 1
# (t/f conv) and phase 2 (affinities); the rest go into phase 2.5
BP1 = 8
BP2 = 24
TFDVE = True   # split tf drains Act/DVE
CONVP = "d2a1"   # conv drain pattern: "alt" (1:1) or "d2a1" (2-of-3 DVE)
IDEN = True     # row pass: fold col-U via identity matmul (else STT add)
PGT = 8         # h-conv Gd staging buffers
PU1 = 4         # col/row Gd read staging buffers
COLP = 3        # col drain: 1-of-COLP blocks on DVE (rest Act)
ROWP = 2        # row drain: 1-of-ROWP halves on DVE (rest Act)
WCND = False    # alternate wcn normalizes onto DVE

_cache = {}


def build_program():
    nc = bacc.Bacc()

    xq_d = nc.dram_tensor("x_q", (128, KC, HW), FP8, kind="ExternalInput")
    tfw_d = nc.dram_tensor("tf_wT", (128, KC, 128), FP8, kind="ExternalInput")
    mw_d = nc.dram_tensor("m_wT", (128, KC, C_OUT), FP8, kind="ExternalInput")
    tfb_d = nc.dram_tensor("tf_b", (128, 1), F32, kind="ExternalInput")
    mask_d = nc.dram_tensor("mask", (96, 96), BF16, kind="ExternalInput")
    ones_d = nc.dram_tensor("ones_s", (96, 128), BF16, kind="ExternalInput")
    iden_d = nc.dram_tensor("iden8", (128, 128), FP8, kind="ExternalInput")
    out_d = nc.dram_tensor("out", (KC, 128, HW), FP8, kind="ExternalOutput")

    with ExitStack() as ctx:
        tc = ctx.enter_context(tile.TileContext(nc))
        p0 = ctx.enter_context(tc.tile_pool(name="p0", bufs=1))

        Xq = p0.tile([128, KC, H, W], FP8)
        ones_s = p0.tile([96, 128], BF16)  # value 1/S2
        mask = p0.tile([96, 96], BF16)
        mw = p0.tile([128, KC, C_OUT], FP8)
        iden8 = p0.tile([128, 128], FP8)

        nc.scalar.dma_start(out=ones_s, in_=ones_d[:])
        nc.scalar.dma_start(out=iden8, in_=iden_d[:])
        nc.scalar.dma_start(out=mask, in_=mask_d[:])
        nc.scalar.dma_start(out=mw, in_=mw_d[:])
        xv = xq_d[:].rearrange("p a (h w) -> p a h w", h=H)

        TF = p0.tile([128, 2, H, W], BF16, tag="big", name="TF")
        T = TF[0:64, 0]
        F = TF[0:64, 1]

        Xflat = Xq.rearrange("p a h w -> p a (h w)")
        TFflat = TF.rearrange("p c h w -> p c (h w)")

        pgd = ctx.enter_context(tc.tile_pool(name="pgd", bufs=1, space="DRAM"))
        Gd = pgd.tile([H, W, C_OUT], FP8)  # h * S1, [y, x, c] (flat pixels)
        Gflat = Gd.rearrange("h w c -> (h w) c")

        with tc.tile_pool(name="pwr", bufs=1) as pwr, \
             tc.tile_pool(name="pwc", bufs=1) as pwc:
            Wr = pwr.tile([96, H, W], BF16)   # exp(row affinity)[i, y, x]
            Wc = pwc.tile([96, W, H], BF16)   # exp(col affinity)[j, x, y]
            wcn = pwc.tile([96, W, H], FP8)   # Wc * S2/D
            wrn = pwr.tile([96, H, W], FP8)   # Wr * S2/D

            hsc = S1 / (S0 * SM)
            NBP = HW // 256  # 36 conv block-pairs

            with tc.tile_pool(name="pgt", bufs=PGT) as pgt, \
                 tc.tile_pool(name="pg_ps", bufs=2, space="PSUM") as pg_ps:

                bp_ctr = [0]

                def emit_conv_bp(bp, drain):
                    """One pair of 128-pixel h-conv blocks -> Gd.
                    Both halves share one PSUM tile; single batched drain."""
                    gtb = pgt.tile([128, 2, C_OUT], FP8, tag="gt")
                    psg = pg_ps.tile([128, 2, C_OUT], F32, tag="pg")
                    for half in range(2):
                        b = 2 * bp + half
                        for t in range(KC // 2):
                            nc.tensor.matmul(
                                psg[:, half, :],
                                Xflat[:, 2 * t:2 * t + 2,
                                      b * 128:(b + 1) * 128],
                                mw[:, 2 * t:2 * t + 2, :],
                                start=(t == 0), stop=(t == KC // 2 - 1),
                                perf_mode=DR)
                    k = bp_ctr[0]
                    bp_ctr[0] += 1
                    if drain == "dve":
                        use_dve = True
                    elif drain == "act":
                        use_dve = False
                    elif drain == "half":
                        use_dve = bool(k % 2)
                    elif drain == "p58":
                        use_dve = (k * 5 % 8 < 5)
                    elif CONVP == "d2a1":
                        use_dve = (k % 3 != 2)
                    else:
                        use_dve = bool(k % 2)
                    if use_dve:
                        nc.vector.tensor_scalar_mul(gtb, psg, hsc)
                    else:
                        nc.scalar.activation(gtb, psg, AF.Copy, scale=hsc)
                    gdv = Gflat[bp * 256:(bp + 1) * 256].rearrange(
                        "(a p) c -> p a c", a=2)
                    nc.scalar.dma_start(out=gdv, in_=gtb)

                # ---- phase 1: t/f conv (fp8 DR); x chunks loaded inline ----
                with tc.tile_pool(name="pe1", bufs=1) as pe1, \
                     tc.tile_pool(name="ptf_ps", bufs=2, space="PSUM") as ptf_ps:
                    tfw = pe1.tile([128, KC, 128], FP8)
                    tfb = pe1.tile([128, 1], F32)
                    nc.scalar.dma_start(out=tfw, in_=tfw_d[:])
                    nc.scalar.dma_start(out=tfb, in_=tfb_d[:])
                    tfbb = bass.AP(tensor=tfb.tensor, offset=tfb.offset,
                                   ap=[tfb.ap[0], [0, 512]])
                    for q in range(8):
                        nc.sync.dma_start(
                            out=Xq[:, :, q * 12:(q + 1) * 12, :],
                            in_=xv[:, :, q * 12:(q + 1) * 12, :])

                    bp_next = 0
                    for b in range(HW // 512):
                        sl = slice(b * 512, (b + 1) * 512)
                        pst = ptf_ps.tile([128, 512], F32, tag="pt")
                        for t in range(KC // 2):
                            nc.tensor.matmul(
                                pst, tfw[:, 2 * t:2 * t + 2, :],
                                Xflat[:, 2 * t:2 * t + 2, sl],
                                start=(t == 0), stop=(t == KC // 2 - 1),
                                perf_mode=DR)
                        if TFDVE and b % 2:
                            nc.vector.scalar_tensor_tensor(
                                TFflat[:, 0, sl], pst, 1.0 / (S0 * STF),
                                tfbb, MUL, ADD)
                        else:
                            nc.scalar.activation(TFflat[:, 0, sl], pst,
                                                 AF.Identity, bias=tfb,
                                                 scale=1.0 / (S0 * STF))
                        if b % 3 == 2:
                            sl3 = slice((b - 2) * 512, (b + 1) * 512)
                            nc.sync.dma_start(out=TFflat[0:64, 1, sl3],
                                              in_=TFflat[64:128, 0, sl3])
                        # conv bp b reads the same pixels tf block b just
                        # used — interleave to fill the DMA-bound head
                        if bp_next < BP1:
                            emit_conv_bp(bp_next, "alt")
                            bp_next += 1

                # ---- phase 2: affinities + exp, conv bps interleaved ----
                with tc.tile_pool(name="pe_ps", bufs=2, space="PSUM") as pe_ps:
                    off = 1
                    for ci, y0 in enumerate(range(0, H, 8)):
                        ps = pe_ps.tile([96, 2, 512], F32, tag="pe")
                        for r in range(8):
                            nc.tensor.matmul(
                                ps[:, r // 4, (r % 4) * 96:(r % 4) * 96 + 96],
                                F[:, y0 + r, :], T[:, y0 + r, :],
                                start=True, stop=True)
                        nc.scalar.activation(
                            Wr[:, y0:y0 + 8, :].rearrange(
                                "i (a b) w -> i a (b w)", a=2),
                            ps[:, :, 0:384], AF.Exp)
                        if bp_next < BP1 + BP2 and bp_next < (
                                BP1 + (ci + 1 + off) * BP2 // 24):
                            emit_conv_bp(bp_next, "alt")
                            bp_next += 1
                    mb8 = bass.AP(tensor=mask.tensor, offset=mask.offset,
                                  ap=[mask.ap[0], [0, 8], mask.ap[1]])
                    for ci, x0 in enumerate(range(0, W, 8)):
                        ps = pe_ps.tile([96, 2, 512], F32, tag="pe")
                        for r in range(8):
                            nc.tensor.matmul(
                                ps[:, r // 4, (r % 4) * 96:(r % 4) * 96 + 96],
                                F[:, :, x0 + r], T[:, :, x0 + r],
                                start=True, stop=True)
                        wcs = Wc[:, x0:x0 + 8, :]
                        nc.scalar.activation(
                            wcs.rearrange("j (a b) y -> j a (b y)", a=2),
                            ps[:, :, 0:384], AF.Exp)
                        nc.vector.tensor_mul(wcs, wcs, mb8)
                        if bp_next < BP1 + BP2 and bp_next < (
                                BP1 + (ci + 13) * BP2 // 24):
                            emit_conv_bp(bp_next, "alt")
                            bp_next += 1

                WrT = Wr.rearrange("i h w -> i w h")
                WcT = Wc.rearrange("j x y -> j y x")

                # ---- phase 2.5: denominators, computed ONCE ([x, y] layout,
                # 2 blocks per PSUM tile) -> rrall; Pool normalizes wcn
                # (gates col pass) then wrn (transposed rrall view).
                # Remaining conv bps interleaved. ----
                rrall = p0.tile([128, W, H], F32)   # S2/D, broadcast, [x, y]
                rrT = rrall.rearrange("p x y -> p y x")
                with tc.tile_pool(name="pd_ps", bufs=2, space="PSUM") as pd_ps:
                    for b8 in range(W // 8):
                        psd = pd_ps.tile([128, 2, 512], F32, tag="pd")
                        for i in range(2):
                            s = slice(b8 * 8 + 4 * i, b8 * 8 + 4 * i + 4)
                            nc.tensor.matmul(psd[:, i, 0:384], ones_s,
                                             Wc[:, s, :],
                                             start=True, stop=False)
                            nc.tensor.matmul(psd[:, i, 0:384], ones_s,
                                             WrT[:, s, :],
                                             start=False, stop=True)
                        s8 = slice(b8 * 8, b8 * 8 + 8)
                        nc.vector.reciprocal_approx_fast(
                            rrall[:, s8, :].rearrange(
                                "p (a b) f -> p a (b f)", a=2),
                            psd[:, :, 0:384])
                        weng = nc.vector if (WCND and b8 % 2) else nc.gpsimd
                        weng.tensor_mul(wcn[:, s8, :], Wc[:, s8, :],
                                        rrall[0:96, s8, :])
                        if bp_next < NBP:
                            emit_conv_bp(bp_next, "act")
                            bp_next += 1
                    for b8 in range(H // 8):
                        s8 = slice(b8 * 8, b8 * 8 + 8)
                        nc.gpsimd.tensor_mul(wrn[:, s8, :], Wr[:, s8, :],
                                             rrT[0:96, s8, :])
                        if bp_next < NBP:
                            emit_conv_bp(bp_next, "act")
                            bp_next += 1
                    while bp_next < NBP:
                        emit_conv_bp(bp_next, "act")
                        bp_next += 1

            # ---- phases 4+5: col pass then row pass, one scope ----
            U = p0.tile([128, KC, H, W], FP8, tag="big", name="U")
            uo = 1.0 / (S1 * S2) * S3   # psum -> U-fp8 scale
            outv = out_d[:].rearrange("k p q -> p k q")
            with tc.tile_pool(name="pu1", bufs=PU1) as pu1, \
                 tc.tile_pool(name="pu2", bufs=8) as pu2:
                with tc.tile_pool(name="pu_ps", bufs=2,
                                  space="PSUM") as pu_ps:
                    for xb in range(W // 4):
                        x0 = xb * 4
                        xs = slice(x0, x0 + 4)
                        gcb = pu1.tile([96, 4, C_OUT], FP8, tag="gc")
                        nc.scalar.dma_start(out=gcb, in_=Gd[:, xs, :])
                        psu = pu_ps.tile([128, 4, 512], F32, tag="pu")
                        for cc in range(4):
                            for r in range(4):
                                nc.tensor.matmul(
                                    psu[:, cc, r * 96:(r + 1) * 96],
                                    gcb[:, r, cc * 128:(cc + 1) * 128],
                                    wcn[:, x0 + r, :],
                                    start=True, stop=True)
                        uv = U[:, :, :, xs]
                        psv = psu[:, :, 0:384].rearrange(
                            "p c (x y) -> p c y x", x=4)
                        if xb * 5 % 12 < 5:
                            nc.vector.tensor_scalar_mul(uv, psv, uo)
                        else:
                            nc.scalar.activation(uv, psv, AF.Copy, scale=uo)

                # row pass: software-pipelined; the identity matmul folds the
                # col-pass U into PSUM so the drain is a plain scaled copy
                with tc.tile_pool(name="pu_ps2", bufs=4,
                                  space="PSUM") as pu_ps2:
                    NYB = H // 4
                    LA = 2
                    tiles = {}

                    def emit_row_mm(yb):
                        y0 = yb * 4
                        rgb = pu2.tile([96, 4, C_OUT], FP8, tag="rg")
                        nc.sync.dma_start(
                            out=rgb, in_=Gd[y0:y0 + 4].rearrange(
                                "y x c -> x y c"))
                        ts = []
                        for ch in range(2):
                            psu = pu_ps2.tile([128, 2, 512], F32, tag="pu2")
                            for c2 in range(2):
                                cc = 2 * ch + c2
                                # start only on r==0: the bank's lazy-zero
                                # covers the disjoint r-regions, so the later
                                # identity accumulate sees clean state
                                for r in range(4):
                                    nc.tensor.matmul(
                                        psu[:, c2, r * 96:(r + 1) * 96],
                                        rgb[:, r, cc * 128:(cc + 1) * 128],
                                        wrn[:, y0 + r, :],
                                        start=(r == 0), stop=False,
                                        skip_group_check=True)
                            ts.append(psu)
                        tiles[yb] = ts

                    def emit_row_fin(yb):
                        y0 = yb * 4
                        ys = slice(y0, y0 + 4)
                        for ch in range(2):
                            psu = tiles[yb][ch]
                            uv = U[:, 2 * ch:2 * ch + 2, ys, :]
                            psv = psu[:, :, 0:384].rearrange(
                                "p c (a b) -> p c a b", a=4)
                            if IDEN and (2 * yb + ch) % ROWP != 1:
                                # Act half: fold col-U into PSUM on PE, then
                                # a plain scaled copy (Act has no tensor add)
                                for c2 in range(2):
                                    cc = 2 * ch + c2
                                    nc.tensor.matmul(
                                        psu[:, c2, 0:384], iden8,
                                        U[:, cc, ys, :],
                                        start=False, stop=True,
                                        skip_group_check=True)
                                nc.scalar.activation(uv, psv, AF.Copy,
                                                     scale=uo)
                            else:
                                # DVE half: STT does the add at copy cost
                                nc.vector.scalar_tensor_tensor(
                                    uv, psv, uo, uv, MUL, ADD)
                        del tiles[yb]
                        if yb % 2 == 1:
                            sl8 = slice((yb - 1) * 4 * 96, (yb + 1) * 4 * 96)
                            nc.scalar.dma_start(
                                out=outv[:, :, sl8],
                                in_=U[:, :, (yb - 1) * 4:(yb + 1) * 4, :])

                    for j in range(NYB + LA):
                        if j < NYB:
                            emit_row_mm(j)
                        if j >= LA:
                            emit_row_fin(j - LA)

    nc.finalize()
    return nc


def _prep_shared(t_w, t_b, f_w, f_b, g_w, g_b, inc_w, inc_b):
    bf = ml_dtypes.bfloat16
    f8 = ml_dtypes.float8_e4m3
    tf_wT = np.concatenate([t_w.T, f_w.T], axis=1)  # (512, 128)
    M = inc_w @ g_w  # (512, 512)
    d = {
        "tf_wT": np.ascontiguousarray(
            (tf_wT * STF).reshape(KC, 128, 128).transpose(1, 0, 2)).astype(f8),
        "m_wT": np.ascontiguousarray(
            (M.T * SM).reshape(KC, 128, C_OUT).transpose(1, 0, 2)).astype(f8),
        "tf_b": np.concatenate([t_b, f_b]).reshape(128, 1).astype(np.float32),
        "mask": (1.0 - np.eye(96)).astype(bf),
        "ones_s": np.full((96, 128), 1.0 / S2, dtype=np.float32).astype(bf),
        "iden8": (np.eye(128, dtype=np.float32) * (S1 * S2 / S3)).astype(f8),
    }
    comb_b = inc_b + inc_w @ g_b
    return d, comb_b


def kernel(x, t_w, t_b, f_w, f_b, g_w, g_b, inc_w, inc_b):
    x = np.asarray(x, dtype=np.float32)
    shared, comb_b = _prep_shared(
        np.asarray(t_w, np.float32), np.asarray(t_b, np.float32),
        np.asarray(f_w, np.float32), np.asarray(f_b, np.float32),
        np.asarray(g_w, np.float32), np.asarray(g_b, np.float32),
        np.asarray(inc_w, np.float32), np.asarray(inc_b, np.float32))

    f8 = ml_dtypes.float8_e4m3
    in_maps = []
    for n in range(N):
        xi = x[n].reshape(KC, 128, HW)  # (4, 128, 9216)
        m = dict(shared)
        m["x_q"] = np.ascontiguousarray(
            xi.transpose(1, 0, 2) * S0).astype(f8)
        in_maps.append(m)

    if "nc" not in _cache:
        _cache["nc"] = build_program()
    res = run_bass_kernel_spmd(_cache["nc"], in_maps, core_ids=list(range(N)))
    attn = np.stack([r["out"].astype(np.float32).reshape(C_IN, H, W)
                     for r in res.results]) * (1.0 / S3)
    return x + attn + comb_b.astype(np.float32)[None, :, None, None]


if __name__ == "__main__":
    rng = np.random.default_rng(0)
    ins = {
        "x": rng.standard_normal((N, C_IN, H, W), dtype=np.float32),
        "t_w": rng.standard_normal((C_INNER, C_IN), dtype=np.float32) * 0.02,
        "t_b": np.zeros(C_INNER, np.float32),
        "f_w": rng.standard_normal((C_INNER, C_IN), dtype=np.float32) * 0.02,
        "f_b": np.zeros(C_INNER, np.float32),
        "g_w": rng.standard_normal((C_OUT, C_IN), dtype=np.float32) * 0.02,
        "g_b": np.zeros(C_OUT, np.float32),
        "inc_w": rng.standard_normal((C_IN, C_OUT), dtype=np.float32) * 0.02,
        "inc_b": np.zeros(C_IN, np.float32),
    }
    y = kernel(**ins)
    print(y.shape, y.dtype)
